# revision 31
# baseline (speedup 1.0000x reference)
"""AttentionRNN Trainium2 kernel: 8-core SPMD, vocab-split fc projection.

Self-contained: kernel(**inputs) takes full inputs, returns full [B,S,V] output.
Strategy: every core runs the identical embed+xproj+RNN+attention program
(replicated; the RNN scan is latency-bound so data-parallelism would not help),
and computes a 1/8 vocab slice of the final fc projection (the dominant cost,
537 GFLOP total). No collectives needed; host concatenates the vocab slices.
All matmuls in bf16 with f32 PSUM accumulation (measured end-to-end rel err
~3.5e-3 vs f32 reference).
"""
import sys
if '/opt/trn_rl_repo' not in sys.path:
    sys.path.insert(0, '/opt/trn_rl_repo')

import numpy as np
import ml_dtypes

import concourse.bass as bass
import concourse.mybir as mybir
import concourse.tile as tile
from concourse import bacc
from concourse.bass_utils import run_bass_kernel_spmd
from concourse.masks import make_identity

DT = mybir.dt
BF = DT.bfloat16
F32 = DT.float32
BF_NP = ml_dtypes.bfloat16

VOCAB, H, B, S = 32000, 512, 16, 512
NCORES = 8
VS = VOCAB // NCORES          # 4000 vocab rows per core
TOK = B * S                   # 8192 tokens, order tok = t*16 + b
KH = H // 128                 # 4 h-chunks
KD = (2 * H) // 128           # 8 d-chunks of combined
FC_VW = 512                   # fc vocab chunk width
NVB = (VS + FC_VW - 1) // FC_VW  # fc vocab chunks per core

# debug dump selector: subset of {"uT", "hsT", "ctxT"}
DEBUG_DUMPS = ()
PHASES = 4


def _vb_width(vb):
    return min(512, VS - vb * 512)


def build_nc(phases=PHASES, dumps=DEBUG_DUMPS, repeat=1):
    nc = bacc.Bacc("TRN2", target_bir_lowering=False, debug=False,
                   num_devices=NCORES)

    emb_bf = nc.dram_tensor("emb_bf", [VOCAB, H], BF, kind="ExternalInput").ap()
    idxw = nc.dram_tensor("idxw", [128, TOK // 16], DT.int16, kind="ExternalInput").ap()
    wxhT = nc.dram_tensor("wxhT", [128, KH * H], BF, kind="ExternalInput").ap()
    whhT = nc.dram_tensor("whhT", [128, KH * H], BF, kind="ExternalInput").ap()
    biasT = nc.dram_tensor("biasT", [128, KH], F32, kind="ExternalInput").ap()
    maskT = nc.dram_tensor("maskT", [128, 128], F32, kind="ExternalInput").ap()
    fcwT = nc.dram_tensor("fcwT", [128, NVB * KD * FC_VW], BF, kind="ExternalInput").ap()
    fcb = nc.dram_tensor("fcb", [128, VS], F32, kind="ExternalInput").ap()
    if phases >= 4:
        y = nc.dram_tensor("y", [B, S, VS], F32, kind="ExternalOutput").ap()
    dump_aps = {}
    for name in dumps:
        dump_aps[name] = nc.dram_tensor(
            name + "_dump", [128, KH * TOK], BF, kind="ExternalOutput").ap()

    NT = 512                  # tok chunk for gather + xproj
    NCH = TOK // NT           # 16 chunks
    NSC = 32                  # RNN steps per streamed u chunk
    NUC = S // NSC            # u chunks
    VW = FC_VW                # fc vocab chunk width
    NVB2 = NVB

    with tile.TileContext(nc) as tc:
      for _rep in range(repeat):
        # u = xproj + biases round-trips through HBM so the RNN phase leaves
        # enough SBUF for the attention/fc pools to coexist (streaming).
        u_dram = nc.dram_tensor(f"u_dram{_rep}", [128, KH * TOK], BF).ap()
        u_dram3 = u_dram.rearrange("p (k n) -> p k n", k=KH)
        with tc.tile_pool(name="perm", bufs=1) as perm:
            hsT = perm.tile([128, KH * TOK], BF, tag="hsT")
            ident = perm.tile([128, 128], BF, tag="ident")
            make_identity(nc, ident[:])

            # [128, KH, TOK] views; free index = t*16+b
            hsT3 = hsT[:].rearrange("p (k n) -> p k n", k=KH)
            hsT4 = hsT[:].rearrange("p (k t b) -> p k t b", k=KH, b=B)
            hsT_t = hsT[:].rearrange("p (k t b) -> p t k b", k=KH, b=B)

            # ---------------- phase 1: gather + xproj (u -> HBM) ----------
            with tc.tile_pool(name="p_x", bufs=1) as p_x, \
                 tc.tile_pool(name="p_u", bufs=3) as p_u:
                xeT = p_x.tile([128, KH * TOK], BF, tag="xeT")
                wxh_sb = p_x.tile([128, KH * H], BF, tag="wxh")
                bias_sb = p_x.tile([128, KH], F32, tag="bias")
                idx_sb = p_x.tile([128, TOK // 16], DT.int16, tag="idx")
                nc.sync.dma_start(out=wxh_sb[:], in_=wxhT[:])
                nc.sync.dma_start(out=bias_sb[:], in_=biasT[:])
                nc.sync.dma_start(out=idx_sb[:], in_=idxw[:])
                # chunk-major gather layout: [p, chunk, k, i] = emb[tok, k*128+p]
                xeT4 = xeT[:].rearrange("p (c k n) -> p c k n", c=NCH, k=KH)

                for c in range(NCH):
                    nc.gpsimd.dma_gather(
                        out_ap=xeT4[:, c],
                        in_ap=emb_bf[:],
                        idxs_ap=idx_sb[:, c * (NT // 16):(c + 1) * (NT // 16)],
                        num_idxs=NT,
                        num_idxs_reg=NT,
                        elem_size=H,
                        transpose=True,
                        single_packet=False,
                    )

                with tc.tile_pool(name="ps_x", bufs=4, space="PSUM") as ps_x:
                    for tci in range(NCH):
                        ustg = p_u.tile([128, KH * NT], BF, tag="ustg")
                        ustg3 = ustg[:].rearrange("p (k n) -> p k n", k=KH)
                        for mg in range(KH):
                            px = ps_x.tile([128, NT], F32, tag="px")
                            for k in range(KH):
                                nc.tensor.matmul(
                                    px[:],
                                    lhsT=wxh_sb[:, k * H + mg * 128:k * H + mg * 128 + 128],
                                    rhs=xeT4[:, tci, k, :],
                                    start=(k == 0), stop=(k == KH - 1),
                                )
                            nc.scalar.activation(
                                ustg3[:, mg], px[:],
                                mybir.ActivationFunctionType.Identity,
                                bias=bias_sb[:, mg:mg + 1],
                            )
                        nc.sync.dma_start(
                            out=u_dram3[:, :, tci * NT:(tci + 1) * NT],
                            in_=ustg3[:, :, :])

            # ---------------- phase 2: RNN scan (u streamed back) ----------
            if phases >= 2:
                with tc.tile_pool(name="p_rnn", bufs=1) as p_rnn, \
                     tc.tile_pool(name="p_ub", bufs=2) as p_ub, \
                     tc.tile_pool(name="ps_r", bufs=1, space="PSUM") as ps_r:
                    whh_sb = p_rnn.tile([128, KH * H], BF, tag="whh")
                    nc.sync.dma_start(out=whh_sb[:], in_=whhT[:])
                    for c in range(NUC):
                        ub = p_ub.tile([128, KH * NSC * B], BF, tag="ub")
                        ub3 = ub[:].rearrange("p (k n) -> p k n", k=KH)
                        nc.scalar.dma_start(
                            out=ub3[:, :, :],
                            in_=u_dram3[:, :, c * NSC * B:(c + 1) * NSC * B])
                        for t in range(c * NSC, (c + 1) * NSC):
                            tl = (t - c * NSC) * B
                            if t == 0:
                                ub_t0 = ub[:].rearrange(
                                    "p (k t b) -> p t k b", k=KH, b=B)
                                nc.scalar.activation(
                                    hsT_t[:, 0], ub_t0[:, 0],
                                    mybir.ActivationFunctionType.Tanh)
                                continue
                            prev = slice((t - 1) * B, t * B)
                            # one psum bank holds all 4 m-chunks [128, 4*16]
                            pm = ps_r.tile([128, KH * B], F32, tag="pr")
                            pm2 = pm[:].rearrange("p (k b) -> p k b", k=KH)
                            for mg in range(KH):
                                nc.tensor.matmul(
                                    pm2[:, mg], lhsT=ident[:],
                                    rhs=ub3[:, mg, tl:tl + B],
                                    start=True, stop=False)
                                for k in range(KH):
                                    nc.tensor.matmul(
                                        pm2[:, mg],
                                        lhsT=whh_sb[:, k * H + mg * 128:k * H + mg * 128 + 128],
                                        rhs=hsT3[:, k, prev],
                                        start=False, stop=(k == KH - 1))
                            nc.scalar.activation(
                                hsT_t[:, t], pm2[:],
                                mybir.ActivationFunctionType.Tanh)

                    if "hsT" in dump_aps:
                        nc.sync.dma_start(out=dump_aps["hsT"][:], in_=hsT[:])

                    # ------- phases 3+4: block-streamed attention + fc -------
                    # tq-blocks of 128 timesteps; block mq only needs hs for
                    # t < (mq+1)*128, so attention + fc for early blocks can
                    # overlap the tail of the RNN (pools coexist with p_rnn).
                    if phases >= 3:
                        TB = 128 * B  # 2048 toks per block
                        with tc.tile_pool(name="ph3", bufs=1) as p3, \
                             tc.tile_pool(name="ctxp", bufs=2) as ctxp, \
                             tc.tile_pool(name="p3w", bufs=2) as p3w, \
                             tc.tile_pool(name="fcw", bufs=2) as pfcw, \
                             tc.tile_pool(name="fco", bufs=2) as pfco, \
                             tc.tile_pool(name="ps_s", bufs=2, space="PSUM") as ps_s, \
                             tc.tile_pool(name="ps_t", bufs=2, space="PSUM") as ps_t, \
                             tc.tile_pool(name="ps_c", bufs=1, space="PSUM") as ps_c, \
                             tc.tile_pool(name="ps_o", bufs=2, space="PSUM") as ps_o:
                            # hs in [tk-part, (chunk, b, h)] layout, per block
                            hs_all = p3.tile([128, KH * B * H], BF, tag="hs_all")
                            hs_all4 = hs_all[:].rearrange(
                                "p (c b h) -> p c b h", c=KH, b=B)
                            mask_sb = p3.tile([128, 128], BF, tag="mask")
                            nc.gpsimd.dma_start(out=mask_sb[:], in_=maskT[:])
                            if phases >= 4:
                                fcb_sb = p3.tile([128, VS], BF, tag="fcb")
                                nc.gpsimd.dma_start(out=fcb_sb[:], in_=fcb[:])
                                fcwT3 = fcwT.rearrange("p (vb x) -> p vb x", vb=NVB)
                                y_r = y.rearrange("b (mt dt) v -> mt dt b v", dt=8)
                            for mq in range(KH):
                                ntk = (mq + 1) * 128
                                ctxb = ctxp.tile([128, KH * TB], BF, tag="ctxb")
                                ctxb3 = ctxb[:].rearrange("p (k n) -> p k n", k=KH)
                                ctxb4 = ctxb[:].rearrange(
                                    "p (k t b) -> p k t b", k=KH, b=B)
                                for b in range(B):
                                    # transpose this block's hs chunk (4 kh)
                                    ptt = ps_t.tile([128, KH * 128], BF, tag="ptt")
                                    ptt3 = ptt[:].rearrange("p (k n) -> p k n", k=KH)
                                    for kh in range(KH):
                                        nc.tensor.transpose(
                                            ptt3[:, kh],
                                            hsT4[:, kh, mq * 128:(mq + 1) * 128, b],
                                            ident[:])
                                    nc.vector.tensor_copy(
                                        hs_all4[:, mq, b, :], ptt[:])
                                    # scores, tk <= ntk only (causal skip)
                                    ps = ps_s.tile([128, S], F32, tag="ps")
                                    for kh in range(KH):
                                        nc.tensor.matmul(
                                            ps[:, 0:ntk],
                                            lhsT=hsT4[:, kh, mq * 128:(mq + 1) * 128, b],
                                            rhs=hsT4[:, kh, 0:ntk, b],
                                            start=(kh == 0), stop=(kh == KH - 1))
                                    # mask diag block in place in psum
                                    nc.vector.tensor_tensor(
                                        out=ps[:, mq * 128:ntk],
                                        in0=ps[:, mq * 128:ntk],
                                        in1=mask_sb[:], op=mybir.AluOpType.add)
                                    st = p3w.tile([128, 4], F32, tag="st")
                                    nmx, zs, zi = st[:, 0:1], st[:, 1:2], st[:, 2:3]
                                    nc.vector.reduce_max(
                                        nmx, ps[:, 0:ntk],
                                        axis=mybir.AxisListType.X, negate=True)
                                    es = p3w.tile([128, S], BF, tag="es")
                                    nc.scalar.activation(
                                        es[:, 0:ntk], ps[:, 0:ntk],
                                        mybir.ActivationFunctionType.Exp,
                                        bias=nmx, accum_out=zs)
                                    nc.vector.reciprocal(zi, zs)
                                    w_sb = p3w.tile([128, S], BF, tag="w_sb")
                                    nc.vector.tensor_scalar_mul(
                                        w_sb[:, 0:ntk], es[:, 0:ntk], zi)
                                    # transpose w chunks -> wT [tk-part, 128 tq]
                                    wT = p3w.tile([128, KH * 128], BF, tag="wT")
                                    wT3 = wT[:].rearrange("p (c n) -> p c n", c=KH)
                                    for ktk in range(mq + 1):
                                        pt = ps_t.tile([128, KH * 128], BF,
                                                       tag="ptt", name="pt")
                                        nc.tensor.transpose(
                                            pt[:, 0:128],
                                            w_sb[:, ktk * 128:(ktk + 1) * 128],
                                            ident[:])
                                        nc.vector.tensor_copy(
                                            wT3[:, ktk, :], pt[:, 0:128])
                                    # contextT block cols for b: [(kh) h, tq]
                                    pc = ps_c.tile([128, KH * 128], F32, tag="pc")
                                    pc3 = pc[:].rearrange("p (k n) -> p k n", k=KH)
                                    for mh in range(KH):
                                        for ktk in range(mq + 1):
                                            nc.tensor.matmul(
                                                pc3[:, mh],
                                                lhsT=hs_all4[:, ktk, b,
                                                             mh * 128:(mh + 1) * 128],
                                                rhs=wT3[:, ktk, :],
                                                start=(ktk == 0), stop=(ktk == mq))
                                    nc.vector.tensor_copy(
                                        ctxb4[:, :, :, b], pc3[:, :, :])
                                # fc for this block's 16 token tiles
                                if phases >= 4:
                                    for vb in range(NVB2):
                                        vw = min(VW, VS - vb * VW)
                                        fw = pfcw.tile([128, KD * VW], BF, tag="fw")
                                        fw3 = fw[:].rearrange(
                                            "p (k v) -> p k v", k=KD)
                                        nc.scalar.dma_start(
                                            out=fw[:], in_=fcwT3[:, vb, :])
                                        for mtl in range(TB // 128):
                                            mt = mq * (TB // 128) + mtl
                                            po = ps_o.tile([128, VW], F32, tag="po")
                                            for k in range(KD):
                                                lhsT = (hsT3[:, k, mt * 128:(mt + 1) * 128]
                                                        if k < KH else
                                                        ctxb3[:, k - KH,
                                                              mtl * 128:(mtl + 1) * 128])
                                                nc.tensor.matmul(
                                                    po[:, 0:vw], lhsT=lhsT,
                                                    rhs=fw3[:, k, 0:vw],
                                                    start=(k == 0), stop=(k == KD - 1))
                                            ob = pfco.tile([128, VW], F32, tag="ob")
                                            nc.vector.tensor_tensor(
                                                out=ob[:, 0:vw], in0=po[:, 0:vw],
                                                in1=fcb_sb[:, vb * VW:vb * VW + vw],
                                                op=mybir.AluOpType.add)
                                            nc.sync.dma_start(
                                                out=y_r[mt, :, :, vb * VW:vb * VW + vw],
                                                in_=ob[:, 0:vw])
    nc.compile()
    return nc


# ---------------------------------------------------------------------------
# host side
# ---------------------------------------------------------------------------

def prep_inputs(x, emb, Wxh_w, Wxh_b, Whh_w, Whh_b, fc_w, fc_b):
    """Build per-core in_maps with device layouts."""
    x = np.asarray(x)
    emb = np.asarray(emb, dtype=np.float32)
    Wxh_w = np.asarray(Wxh_w, dtype=np.float32)
    Wxh_b = np.asarray(Wxh_b, dtype=np.float32)
    Whh_w = np.asarray(Whh_w, dtype=np.float32)
    Whh_b = np.asarray(Whh_b, dtype=np.float32)
    fc_w = np.asarray(fc_w, dtype=np.float32)
    fc_b = np.asarray(fc_b, dtype=np.float32)

    emb_bf = np.ascontiguousarray(emb.astype(BF_NP))
    # idx wrapped: flat tok order = t*16+b ; slot j -> [j%16, j//16]
    idx_flat = np.ascontiguousarray(x.T).reshape(-1).astype(np.int64)  # [S*B] t-major
    wrapped = idx_flat.reshape(TOK // 16, 16).T.astype(np.int16)  # [16, TOK//16]
    # replicated across the 8 gpsimd Q7 cores: each reads its own 16-partition group
    idxw = np.ascontiguousarray(np.tile(wrapped, (8, 1)))

    def pack_T(w):  # w [G, H] -> lhsT layout [128, KH*G] : [p, k*G+g] = w[g, k*128+p]
        wT = np.ascontiguousarray(w.T)            # [H, G]
        kh = wT.shape[0] // 128
        return np.ascontiguousarray(
            wT.reshape(kh, 128, wT.shape[1]).transpose(1, 0, 2).reshape(128, -1)
        ).astype(BF_NP)

    wxhT = pack_T(Wxh_w)                          # [128, KH*H]
    whhT = pack_T(Whh_w)
    bias = (Wxh_b + Whh_b).astype(np.float32)
    biasT = np.ascontiguousarray(bias.reshape(KH, 128).T)  # [128, KH]

    p = np.arange(128)[:, None]
    j = np.arange(128)[None, :]
    maskT = np.where(j <= p, 0.0, -1e30).astype(np.float32)

    base = {
        "emb_bf": emb_bf, "idxw": idxw, "wxhT": wxhT, "whhT": whhT,
        "biasT": biasT, "maskT": maskT,
    }
    in_maps = []
    for c in range(NCORES):
        sl = slice(c * VS, (c + 1) * VS)
        fcwT_kv = pack_T(fc_w[sl]).reshape(128, KD, VS)   # [p, k, v]
        # vb-major contiguous: [p, vb, k, FC_VW] (zero-padded last chunk)
        fcwT = np.zeros((128, NVB, KD, FC_VW), BF_NP)
        for vb in range(NVB):
            vw = min(FC_VW, VS - vb * FC_VW)
            fcwT[:, vb, :, :vw] = fcwT_kv[:, :, vb * FC_VW:vb * FC_VW + vw]
        fcwT = np.ascontiguousarray(fcwT.reshape(128, NVB * KD * FC_VW))
        fcb_bc = np.ascontiguousarray(
            np.broadcast_to(fc_b[sl].astype(np.float32), (128, VS)))
        m = dict(base)
        m["fcwT"] = fcwT
        m["fcb"] = fcb_bc
        in_maps.append(m)
    return in_maps


_NC_CACHE = {}


def get_nc(phases=PHASES, dumps=DEBUG_DUMPS):
    key = (phases, tuple(dumps))
    if key not in _NC_CACHE:
        _NC_CACHE[key] = build_nc(phases, dumps)
    return _NC_CACHE[key]


def kernel(x, emb, Wxh_w, Wxh_b, Whh_w, Whh_b, fc_w, fc_b):
    nc = get_nc()
    in_maps = prep_inputs(x, emb, Wxh_w, Wxh_b, Whh_w, Whh_b, fc_w, fc_b)
    res = run_bass_kernel_spmd(nc, in_maps, list(range(NCORES)))
    y = np.concatenate([res.results[c]["y"] for c in range(NCORES)], axis=2)
    return np.ascontiguousarray(y.astype(np.float32))
